# revision 5
# baseline (speedup 1.0000x reference)
import os
import numpy as np
import ml_dtypes

import concourse.bass as bass
import concourse.bacc as bacc
import concourse.tile as tile
from concourse import mybir
from concourse.masks import make_identity
from concourse.bass_utils import run_bass_kernel_spmd

F32 = mybir.dt.float32
BF16 = mybir.dt.bfloat16
I32 = mybir.dt.int32
ALU = mybir.AluOpType
ACT = mybir.ActivationFunctionType
AX = mybir.AxisListType
BF = ml_dtypes.bfloat16

B, S, D = 4, 4096, 2048
NH, V, HD = 24, 50000, 64
EH = NH * HD
NCORES = 8
SC = S // NCORES            # 512 tokens per core per batch
HALO = 12                   # (K-1)*DIL causal halo for the short conv
SEG = SC + HALO             # 524
ROWS = B * SEG              # 2096
RPAD = 2176                 # 17 * 128
NT = RPAD // 128
EPS = 1.1920929e-07
SHIFTS = (4, 8, 12)

LAST_RESULT = None


def _out_runs(ti):
    res = []
    lo_t, hi_t = 128 * ti, 128 * ti + 128
    for b in range(B):
        lo = max(lo_t, SEG * b + HALO)
        hi = min(hi_t, SEG * b + SEG)
        if lo < hi:
            res.append((lo - lo_t, hi - lo, b, lo - SEG * b - HALO))
    return res


def _build_nc():
    nc = bacc.Bacc(None)
    hsw_d = nc.dram_tensor("hsw", [RPAD, D], BF16, kind="ExternalInput")
    qv_d = nc.dram_tensor("qv", [RPAD, 2], F32, kind="ExternalInput")
    fids_d = nc.dram_tensor("fids", [RPAD, NH], I32, kind="ExternalInput")
    table_d = nc.dram_tensor("table", [NH * V, HD], BF16, kind="ExternalInput")
    wkt_d = nc.dram_tensor("wkt", [EH, D], BF16, kind="ExternalInput")
    wvt_d = nc.dram_tensor("wvt", [EH, D], BF16, kind="ExternalInput")
    bk_d = nc.dram_tensor("bkr", [1, D], BF16, kind="ExternalInput")
    bv_d = nc.dram_tensor("bvr", [1, D], BF16, kind="ExternalInput")
    wc_d = nc.dram_tensor("wc", [4 * 128, D], BF16, kind="ExternalInput")
    out_d = nc.dram_tensor("out", [B, SC, D], F32, kind="ExternalOutput")

    with tile.TileContext(nc) as tc:
        with tc.tile_pool(name="const", bufs=1) as cp, \
             tc.tile_pool(name="hp", bufs=2) as hp, \
             tc.tile_pool(name="qp", bufs=2) as qp, \
             tc.tile_pool(name="fp", bufs=2) as fp, \
             tc.tile_pool(name="ep", bufs=2) as ep, \
             tc.tile_pool(name="vp", bufs=2) as vp, \
             tc.tile_pool(name="op", bufs=1) as op, \
             tc.tile_pool(name="shp", bufs=2) as shp, \
             tc.tile_pool(name="prp", bufs=2) as prp, \
             tc.tile_pool(name="rp", bufs=2) as rp, \
             tc.tile_pool(name="tpp", bufs=2, space="PSUM") as tpp, \
             tc.tile_pool(name="kpp", bufs=2, space="PSUM") as kpp, \
             tc.tile_pool(name="vpp", bufs=2, space="PSUM") as vpp:
            wkt_t = []
            for i in range(12):
                t = cp.tile([128, D], BF16, name=f"wkt{i}")
                nc.sync.dma_start(out=t[:], in_=wkt_d[128 * i:128 * i + 128, :])
                wkt_t.append(t)
            wvt_t = []
            for i in range(12):
                t = cp.tile([128, D], BF16, name=f"wvt{i}")
                nc.sync.dma_start(out=t[:], in_=wvt_d[128 * i:128 * i + 128, :])
                wvt_t.append(t)
            wc_t = []
            for i in range(4):
                t = cp.tile([128, D], BF16, name=f"wc{i}")
                nc.sync.dma_start(out=t[:], in_=wc_d[128 * i:128 * i + 128, :])
                wc_t.append(t)
            bk_t = cp.tile([1, D], BF16)
            nc.sync.dma_start(out=bk_t[:], in_=bk_d[:])
            bv_t = cp.tile([1, D], BF16)
            nc.sync.dma_start(out=bv_t[:], in_=bv_d[:])
            ident = cp.tile([128, 128], BF16)
            make_identity(nc, ident[:])
            ones1 = cp.tile([1, 128], BF16)
            nc.vector.memset(ones1[:], 1.0)
            sq_t = cp.tile([128, 512], F32)
            epsc = cp.tile([128, 1], F32)
            nc.vector.memset(epsc[:], EPS)

            vprev = vp.tile([128, D], F32)
            nc.vector.memset(vprev[:], 0.0)

            for ti in range(NT):
                r0 = 128 * ti
                hswt = hp.tile([128, D], BF16)
                nc.sync.dma_start(out=hswt[:], in_=hsw_d[r0:r0 + 128, :])
                qvt = qp.tile([128, 2], F32)
                nc.sync.dma_start(out=qvt[:], in_=qv_d[r0:r0 + 128, :])
                fidst = fp.tile([128, NH], I32)
                nc.sync.dma_start(out=fidst[:], in_=fids_d[r0:r0 + 128, :])

                emb = ep.tile([128, EH], BF16)
                for h in range(NH):
                    nc.gpsimd.indirect_dma_start(
                        out=emb[:, HD * h:HD * h + HD], out_offset=None,
                        in_=table_d[:],
                        in_offset=bass.IndirectOffsetOnAxis(
                            ap=fidst[:, h:h + 1], axis=0))
                # in-place transpose: emb chunk i becomes embT chunk i
                for i in range(12):
                    tp = tpp.tile([128, 128], BF16)
                    nc.tensor.transpose(
                        out=tp[:], in_=emb[:, 128 * i:128 * i + 128],
                        identity=ident[:])
                    nc.scalar.activation(
                        out=emb[:, 128 * i:128 * i + 128], in_=tp[:],
                        func=ACT.Copy)

                red = rp.tile([128, 16], F32)
                nc.vector.memset(red[:], 0.0)
                for c4 in range(4):
                    kp = kpp.tile([128, 512], F32)
                    for i in range(12):
                        nc.tensor.matmul(
                            out=kp[:], lhsT=emb[:, 128 * i:128 * i + 128],
                            rhs=wkt_t[i][:, 512 * c4:512 * c4 + 512],
                            start=(i == 0), stop=False)
                    nc.tensor.matmul(
                        out=kp[:], lhsT=ones1[:],
                        rhs=bk_t[:, 512 * c4:512 * c4 + 512],
                        start=False, stop=True)
                    nc.scalar.activation(
                        out=sq_t[:], in_=kp[:], func=ACT.Square,
                        accum_out=red[:, c4:c4 + 1])
                    pr = prp.tile([128, 512], F32)
                    nc.vector.tensor_tensor(
                        out=pr[:], in0=kp[:],
                        in1=hswt[:, 512 * c4:512 * c4 + 512], op=ALU.mult)
                    nc.vector.reduce_sum(
                        out=red[:, 4 + c4:5 + c4], in_=pr[:], axis=AX.X)

                # gate chain (all [128,1])
                nc.vector.reduce_sum(out=red[:, 8:9], in_=red[:, 0:4],
                                     axis=AX.X)
                nc.vector.reduce_sum(out=red[:, 9:10], in_=red[:, 4:8],
                                     axis=AX.X)
                nc.scalar.activation(out=red[:, 10:11], in_=red[:, 8:9],
                                     func=ACT.Ln, bias=epsc[:], scale=1.0 / D)
                nc.scalar.activation(out=red[:, 11:12], in_=red[:, 10:11],
                                     func=ACT.Exp, scale=-0.5)
                nc.vector.tensor_tensor(out=red[:, 12:13], in0=red[:, 9:10],
                                        in1=red[:, 11:12], op=ALU.mult)
                nc.vector.tensor_tensor(out=red[:, 12:13], in0=red[:, 12:13],
                                        in1=qvt[:, 0:1], op=ALU.mult)
                nc.scalar.activation(out=red[:, 13:14], in_=red[:, 12:13],
                                     func=ACT.Abs)
                nc.vector.tensor_scalar(out=red[:, 13:14], in0=red[:, 13:14],
                                        scalar1=1e-6, scalar2=None,
                                        op0=ALU.max)
                nc.scalar.activation(out=red[:, 14:15], in_=red[:, 13:14],
                                     func=ACT.Ln)
                nc.scalar.activation(out=red[:, 13:14], in_=red[:, 14:15],
                                     func=ACT.Exp, scale=-0.5)
                nc.vector.tensor_tensor(out=red[:, 12:13], in0=red[:, 12:13],
                                        in1=red[:, 13:14], op=ALU.mult)
                nc.scalar.activation(out=red[:, 14:15], in_=red[:, 12:13],
                                     func=ACT.Exp, scale=-1.0)
                nc.vector.tensor_scalar(out=red[:, 14:15], in0=red[:, 14:15],
                                        scalar1=1.0, scalar2=None, op0=ALU.add)
                nc.vector.reciprocal(out=red[:, 15:16], in_=red[:, 14:15])
                nc.vector.tensor_tensor(out=red[:, 15:16], in0=red[:, 15:16],
                                        in1=qvt[:, 1:2], op=ALU.mult)

                val = vp.tile([128, D], F32)
                for c4 in range(4):
                    vps = vpp.tile([128, 512], F32)
                    for i in range(12):
                        nc.tensor.matmul(
                            out=vps[:], lhsT=emb[:, 128 * i:128 * i + 128],
                            rhs=wvt_t[i][:, 512 * c4:512 * c4 + 512],
                            start=(i == 0), stop=False)
                    nc.tensor.matmul(
                        out=vps[:], lhsT=ones1[:],
                        rhs=bv_t[:, 512 * c4:512 * c4 + 512],
                        start=False, stop=True)
                    nc.vector.tensor_scalar(
                        out=val[:, 512 * c4:512 * c4 + 512], in0=vps[:],
                        scalar1=red[:, 15:16], scalar2=None, op0=ALU.mult)

                # causal dilated depthwise conv: out = val*c3p1 + sum shifts
                outt = op.tile([128, D], F32)
                nc.gpsimd.tensor_tensor(out=outt[:], in0=val[:],
                                        in1=wc_t[0][:], op=ALU.mult)
                for j, sft in enumerate(SHIFTS):
                    sh = shp.tile([128, D], F32)
                    nc.sync.dma_start(out=sh[sft:128, :],
                                      in_=val[0:128 - sft, :])
                    nc.sync.dma_start(out=sh[0:sft, :],
                                      in_=vprev[128 - sft:128, :])
                    nc.gpsimd.tensor_tensor(out=sh[:], in0=sh[:],
                                            in1=wc_t[1 + j][:], op=ALU.mult)
                    nc.vector.tensor_tensor(out=outt[:], in0=outt[:],
                                            in1=sh[:], op=ALU.add)
                vprev = val

                for (ro, ln, b, t0) in _out_runs(ti):
                    nc.sync.dma_start(out=out_d[b, t0:t0 + ln, :],
                                      in_=outt[ro:ro + ln, :])
    nc.finalize()
    return nc


def kernel(**inputs):
    global LAST_RESULT
    hs = np.asarray(inputs["hidden_states"], np.float32)
    hid = np.asarray(inputs["hash_ids"], np.int32)
    emb_table = np.asarray(inputs["emb_table"], np.float32)
    Wk = np.asarray(inputs["Wk"], np.float32)
    bk = np.asarray(inputs["bk"], np.float32)
    Wv = np.asarray(inputs["Wv"], np.float32)
    bv = np.asarray(inputs["bv"], np.float32)
    knw = np.asarray(inputs["key_norm_w"], np.float32)
    qnw = np.asarray(inputs["query_norm_w"], np.float32)
    conv_w = np.asarray(inputs["conv_w"], np.float32)
    conv_b = np.asarray(inputs["conv_b"], np.float32)

    hs_w = (hs * (knw * qnw)[None, None, :]).astype(BF)
    ms = np.mean(np.square(hs), axis=-1, dtype=np.float32)
    qfac = (1.0 / (np.sqrt(ms + EPS) * np.sqrt(np.float32(D)))).astype(
        np.float32)
    fids = hid + (np.arange(NH, dtype=np.int32) * V)[None, None, :]
    table = emb_table.reshape(NH * V, HD).astype(BF)
    wkt = np.ascontiguousarray(Wk.T).astype(BF)
    wvt = np.ascontiguousarray(Wv.T).astype(BF)
    bk_b = bk.reshape(1, D).astype(BF)
    bv_b = bv.reshape(1, D).astype(BF)
    wc = np.empty((4 * 128, D), BF)
    wc[0:128] = np.broadcast_to((1.0 + conv_w[:, 3]).astype(BF)[None, :],
                                (128, D))
    for j, col in enumerate((2, 1, 0)):
        wc[128 * (1 + j):128 * (2 + j)] = np.broadcast_to(
            conv_w[:, col].astype(BF)[None, :], (128, D))

    in_maps = []
    for c in range(NCORES):
        hsw_s = np.zeros((RPAD, D), BF)
        qv_s = np.zeros((RPAD, 2), np.float32)
        fid_s = np.zeros((RPAD, NH), np.int32)
        t0 = SC * c
        for b in range(B):
            base = SEG * b
            hsw_s[base + HALO:base + SEG] = hs_w[b, t0:t0 + SC]
            qv_s[base + HALO:base + SEG, 0] = qfac[b, t0:t0 + SC]
            qv_s[base + HALO:base + SEG, 1] = 1.0
            fid_s[base + HALO:base + SEG] = fids[b, t0:t0 + SC]
            if c > 0:
                hsw_s[base:base + HALO] = hs_w[b, t0 - HALO:t0]
                qv_s[base:base + HALO, 0] = qfac[b, t0 - HALO:t0]
                qv_s[base:base + HALO, 1] = 1.0
                fid_s[base:base + HALO] = fids[b, t0 - HALO:t0]
        in_maps.append(dict(hsw=hsw_s, qv=qv_s, fids=fid_s, table=table,
                            wkt=wkt, wvt=wvt, bkr=bk_b, bvr=bv_b, wc=wc))

    nc = _build_nc()
    trace = os.environ.get("KTRACE", "0") == "1"
    try:
        res = run_bass_kernel_spmd(nc, in_maps, list(range(NCORES)),
                                   trace=trace)
    except ModuleNotFoundError:
        if not trace:
            raise
        res = run_bass_kernel_spmd(nc, in_maps, list(range(NCORES)),
                                   trace=False)
    LAST_RESULT = res
    parts = [np.asarray(res.results[c]["out"], np.float32)
             for c in range(NCORES)]
    out = np.concatenate(parts, axis=1)
    out = out + conv_b[None, None, :]
    return out.astype(np.float32)
